# revision 47
# baseline (speedup 1.0000x reference)
"""Trainium2 Bass kernel: single-step attention decoder (embed -> additive
attention -> combine -> 2-layer GRU -> vocab projection + log_softmax).

Sharding across 8 NeuronCores:
  - out_W is sharded over the vocab dim (6250 rows/core).
  - comb_W / GRU weights are sharded over the output (hidden) dim
    (128-wide slice per core); full activations are rebuilt with tiny
    AllGathers (4KB) after comb and after each GRU layer.
  - attention (tiny) is replicated on every core.
  - emb_W is never shipped: the host gathers the single needed row.
  - final log_softmax: per-core sum(exp(logits_shard)) + scalar AllReduce,
    subtract log(S) on device; host concatenates vocab shards.
"""

import os
import sys

import numpy as np

for _p in ("/opt/trn_rl_repo",):
    if os.path.isdir(_p) and _p not in sys.path:
        sys.path.insert(0, _p)

M = 8          # cores
H = 1024       # hidden
V = 50000      # vocab
ML = 50        # encoder slots
L = 2          # gru layers
P = 128        # partitions
KC = H // P    # 8 k-chunks of the hidden dim
VS = V // M    # 6250 vocab rows per core
VT = 512       # vocab tile width (matmul moving free dim)
NV = (VS + VT - 1) // VT       # 13 vocab tiles per core
VTAIL = VS - (NV - 1) * VT     # 106
VPAD = NV * VT                 # 6656
NCOL = VPAD // P               # 52 columns of the [128, NCOL] logits layout
NEG = -1.0e30

_prog = None


def _build(use_collectives=True, nv=NV, skip_chain=False, rounds=1,
           stream_bufs=36, psv_bufs=2, stream_mode="bf16x3"):
    import concourse.mybir as mybir
    import concourse.tile as tile
    from concourse import bacc

    f32 = mybir.dt.float32
    AF = mybir.ActivationFunctionType

    nc = bacc.Bacc("TRN2", target_bir_lowering=False, debug=False, num_devices=M)

    bf16 = mybir.dt.bfloat16
    f32r = {"f32r": mybir.dt.float32r, "f32": f32, "bf16x3": bf16}[stream_mode]

    def din(name, shape, dt=None):
        return nc.dram_tensor(
            name, list(shape), dt or f32, kind="ExternalInput"
        ).ap()

    def dout(name, shape):
        return nc.dram_tensor(name, list(shape), f32, kind="ExternalOutput").ap()

    # bf16x3: weights stored as a bf16 (hi, lo) pair — same 4 bytes/element as
    # f32 — and the matvec runs as 3 bf16 streams (xh*Whi + xl*Whi + xh*Wlo),
    # exact to ~2^-16 while streaming the PE at 1 row/cycle (f32 pays 4x).
    if stream_mode == "bf16x3":
        wt = din("wt", (NV, KC, P, 2 * VT), bf16)  # [v, kc, k, hi|lo]
    else:
        wt = din("wt", (NV, KC, P, VT), f32r)    # [v, kc, k, n]
    gw = din("gw", (P, L * 2 * 3 * KC, P))   # [k, (l,mat,gate,kc), m]
    gb = din("gb", (P, L, 4))                # [slice, l, {br, bz, bin, bhn}]
    cw = din("cw", (P, 2 * KC, P))           # [k, kc, m] comb weight slice
    cb = din("cb", (P, 1))
    aw = din("aw", (P, 2 * KC, ML))          # [k, kc, m] attn weight (replicated)
    ab = din("ab", (ML, 1))
    enc = din("enc", (ML, KC, P))            # [k(slot), mc, m] encoder outputs
    emb = din("emb", (P, KC))                # gathered embedding row, column-spread
    h0 = din("h0", (P, KC))                  # hidden[0] column-spread
    h1 = din("h1", (P, KC))                  # hidden[1] column-spread
    hown = din("hown", (P, L))               # this core's own h slices
    outb = din("outb", (1, 2 * VPAD), bf16)  # out_b shard, bf16 hi||lo row

    o_log = dout("out_logits", (1, VPAD))
    o_newh = dout("out_newh", (L, P))
    o_attw = dout("out_attw", (ML, 1))

    groups = [list(range(M))]

    def gidx(l, mat, g, kc):
        return ((l * 2 + mat) * 3 + g) * KC + kc

    with tile.TileContext(nc) as tc:
        with (
            tc.tile_pool(name="consts", bufs=1) as consts,
            tc.tile_pool(name="wpool", bufs=1) as wpool,
            tc.tile_pool(name="acts", bufs=2) as acts,
            tc.tile_pool(name="tmp", bufs=3) as tmp,
            tc.tile_pool(name="stream", bufs=stream_bufs) as stream,
            tc.tile_pool(name="rows", bufs=4) as rows,
            tc.tile_pool(name="rowbig", bufs=1) as rowbig,
            tc.tile_pool(name="ps_small", bufs=4, space="PSUM") as ps_small,
            tc.tile_pool(name="ps_big", bufs=psv_bufs, space="PSUM") as ps_big,
            tc.tile_pool(name="ps_one", bufs=2, space="PSUM") as ps_one,
            tc.tile_pool(name="dram", bufs=1, space="DRAM") as dram,
        ):
            # ---- constants ----
            ones_col = consts.tile([P, 1], f32, tag="ones_col")
            nc.vector.memset(ones_col, 1.0)
            ones_row = consts.tile([1, P], f32, tag="ones_row")
            nc.vector.memset(ones_row, 1.0)
            ones_bf = consts.tile([1, 1], bf16, tag="ones_bf")
            nc.vector.memset(ones_bf, 1.0)

            # ---- small weights / activations to SBUF (all contiguous DMAs) ----
            aw_sb = wpool.tile([P, 2 * KC, ML], f32, tag="aw")
            nc.sync.dma_start(out=aw_sb, in_=aw)
            enc_sb = wpool.tile([ML, KC, P], f32, tag="enc")
            nc.sync.dma_start(out=enc_sb, in_=enc)
            cw_sb = wpool.tile([P, 2 * KC, P], f32, tag="cw")
            nc.sync.dma_start(out=cw_sb, in_=cw)
            gw_sb = wpool.tile([P, L * 2 * 3 * KC, P], f32, tag="gw")
            for l in range(L):
                for mat in range(2):
                    lo = gidx(l, mat, 0, 0)
                    hi = gidx(l, mat, 2, KC - 1) + 1
                    nc.sync.dma_start(out=gw_sb[:, lo:hi, :], in_=gw[:, lo:hi, :])
            gb_sb = wpool.tile([P, L, 4], f32, tag="gb")
            nc.sync.dma_start(out=gb_sb, in_=gb)
            cb_sb = wpool.tile([P, 1], f32, tag="cb")
            nc.sync.dma_start(out=cb_sb, in_=cb)
            ab_sb = wpool.tile([ML, 1], f32, tag="ab")
            nc.sync.dma_start(out=ab_sb, in_=ab)
            outb_sb = wpool.tile([1, 2 * VPAD], bf16, tag="outb")
            nc.sync.dma_start(out=outb_sb, in_=outb)
            emb_sb = acts.tile([P, KC], f32, tag="emb")
            nc.sync.dma_start(out=emb_sb, in_=emb)
            h0_sb = acts.tile([P, KC], f32, tag="h0")
            nc.sync.dma_start(out=h0_sb, in_=h0)
            h1_sb = acts.tile([P, KC], f32, tag="h1")
            nc.sync.dma_start(out=h1_sb, in_=h1)
            hown_sb = acts.tile([P, L], f32, tag="hown")
            nc.sync.dma_start(out=hown_sb, in_=hown)



            def allgather_vec(slice_sb, name):
                """AllGather this core's [128,1] slice -> full column-spread [128, KC]."""
                ag_in = dram.tile([1, P], f32, tag=f"agin_{name}")
                ag_out = dram.tile([1, P * M], f32, tag=f"agout_{name}")
                nc.sync.dma_start(out=ag_in.transpose([1, 0]), in_=slice_sb)
                if use_collectives:
                    nc.gpsimd.collective_compute(
                        "AllGather",
                        mybir.AluOpType.bypass,
                        replica_groups=groups,
                        ins=[ag_in.opt()],
                        outs=[ag_out.opt()],
                    )
                else:
                    for jj in range(M):
                        nc.sync.dma_start(
                            out=ag_out[:, P * jj : P * (jj + 1)], in_=ag_in
                        )
                full_sb = acts.tile([P, KC], f32, tag=f"agsb_{name}")
                nc.sync.dma_start(
                    out=full_sb,
                    in_=ag_out.rearrange("a (c p) -> a c p", p=P).transpose([2, 1, 0]).opt(),
                )
                return full_sb

            # ---- 2-layer GRU (this core's 128-slice per layer) ----
            def gru_layer(l, x_chunks, h_chunks, hown_col):
                r_ps = ps_small.tile([P, 1], f32, tag="ps")
                z_ps = ps_small.tile([P, 1], f32, tag="ps")
                in_ps = ps_small.tile([P, 1], f32, tag="ps")
                hn_ps = ps_small.tile([P, 1], f32, tag="ps")
                for g, dst in ((0, r_ps), (1, z_ps)):
                    for kc in range(2 * KC):
                        if kc < KC:
                            lhsT = gw_sb[:, gidx(l, 0, g, kc), :]
                            rhs = x_chunks[:, kc : kc + 1]
                        else:
                            lhsT = gw_sb[:, gidx(l, 1, g, kc - KC), :]
                            rhs = h_chunks[:, kc - KC : kc - KC + 1]
                        nc.tensor.matmul(
                            dst, lhsT, rhs, start=(kc == 0), stop=(kc == 2 * KC - 1)
                        )
                for kc in range(KC):
                    nc.tensor.matmul(
                        in_ps,
                        gw_sb[:, gidx(l, 0, 2, kc), :],
                        x_chunks[:, kc : kc + 1],
                        start=(kc == 0),
                        stop=(kc == KC - 1),
                    )
                for kc in range(KC):
                    nc.tensor.matmul(
                        hn_ps,
                        gw_sb[:, gidx(l, 1, 2, kc), :],
                        h_chunks[:, kc : kc + 1],
                        start=(kc == 0),
                        stop=(kc == KC - 1),
                    )
                r_sb = tmp.tile([P, 1], f32, tag="r")
                nc.scalar.activation(
                    out=r_sb, in_=r_ps, func=AF.Sigmoid, bias=gb_sb[:, l, 0:1], scale=1.0
                )
                z_sb = tmp.tile([P, 1], f32, tag="z")
                nc.scalar.activation(
                    out=z_sb, in_=z_ps, func=AF.Sigmoid, bias=gb_sb[:, l, 1:2], scale=1.0
                )
                hnb_sb = tmp.tile([P, 1], f32, tag="hnb")
                nc.vector.tensor_add(hnb_sb, hn_ps, gb_sb[:, l, 3:4])
                rh_sb = tmp.tile([P, 1], f32, tag="rh")
                nc.vector.tensor_mul(rh_sb, r_sb, hnb_sb)
                t_sb = tmp.tile([P, 1], f32, tag="t")
                nc.vector.tensor_add(t_sb, in_ps, rh_sb)
                n_sb = tmp.tile([P, 1], f32, tag="n")
                nc.scalar.activation(
                    out=n_sb, in_=t_sb, func=AF.Tanh, bias=gb_sb[:, l, 2:3], scale=1.0
                )
                d_sb = tmp.tile([P, 1], f32, tag="d")
                nc.vector.tensor_sub(d_sb, hown_col, n_sb)
                zd_sb = tmp.tile([P, 1], f32, tag="zd")
                nc.vector.tensor_mul(zd_sb, z_sb, d_sb)
                hp_sb = tmp.tile([P, 1], f32, tag="hp")
                nc.vector.tensor_add(hp_sb, n_sb, zd_sb)
                nc.sync.dma_start(
                    out=o_newh[l : l + 1, :].transpose([1, 0]), in_=hp_sb
                )
                return hp_sb, allgather_vec(hp_sb, f"h{l}")

            def attention_comb(h0s):
                # att logits, column-spread [50, 1]
                att_ps = ps_small.tile([ML, 1], f32, tag="ps")
                for kc in range(2 * KC):
                    rhs = emb_sb[:, kc : kc + 1] if kc < KC else h0s[:, kc - KC : kc - KC + 1]
                    nc.tensor.matmul(
                        att_ps,
                        aw_sb[:, kc, :],
                        rhs,
                        start=(kc == 0),
                        stop=(kc == 2 * KC - 1),
                    )
                # e = exp(logits + attn_b); softmax denominator via ones-matmul
                e_sb = tmp.tile([ML, 1], f32, tag="e")
                nc.scalar.activation(
                    out=e_sb, in_=att_ps, func=AF.Exp, bias=ab_sb, scale=1.0
                )
                s_ps = ps_small.tile([1, 1], f32, tag="ps")
                nc.tensor.matmul(s_ps, e_sb, ones_col[:ML, :], start=True, stop=True)
                recip_sb = tmp.tile([1, 1], f32, tag="recip")
                nc.vector.reciprocal(recip_sb, s_ps)
                # broadcast 1/s to 128 partitions
                r128_ps = ps_small.tile([P, 1], f32, tag="ps")
                nc.tensor.matmul(r128_ps, ones_row, recip_sb, start=True, stop=True)
                r128_sb = tmp.tile([P, 1], f32, tag="r128")
                nc.scalar.copy(out=r128_sb, in_=r128_ps)
                # att_w output (only core 0's is used by the host)
                attw_sb = tmp.tile([ML, 1], f32, tag="attw")
                nc.vector.tensor_scalar_mul(attw_sb, e_sb, r128_sb[:ML, :])
                nc.sync.dma_start(out=o_attw, in_=attw_sb)
                # att_applied (column-spread [128, KC]) = (e @ enc) / s
                attap_ps = ps_small.tile([P, KC], f32, tag="ps")
                for mc in range(KC):
                    nc.tensor.matmul(
                        attap_ps[:, mc : mc + 1],
                        enc_sb[:, mc, :],
                        e_sb,
                        start=True,
                        stop=True,
                    )
                xatt_sb = acts.tile([P, KC], f32, tag="xatt")
                nc.vector.tensor_scalar_mul(xatt_sb, attap_ps, r128_sb)

                # combine + relu (this core's 128-slice)
                xc_ps = ps_small.tile([P, 1], f32, tag="ps")
                for kc in range(2 * KC):
                    rhs = (
                        emb_sb[:, kc : kc + 1]
                        if kc < KC
                        else xatt_sb[:, kc - KC : kc - KC + 1]
                    )
                    nc.tensor.matmul(
                        xc_ps,
                        cw_sb[:, kc, :],
                        rhs,
                        start=(kc == 0),
                        stop=(kc == 2 * KC - 1),
                    )
                xsl_sb = tmp.tile([P, 1], f32, tag="xsl")
                nc.scalar.activation(
                    out=xsl_sb, in_=xc_ps, func=AF.Relu, bias=cb_sb, scale=1.0
                )
                return xsl_sb

            def chain(h0s, h1s, hown0, hown1):
                xsl_sb = attention_comb(h0s)
                x_cs = allgather_vec(xsl_sb, "x")
                hp0, h0p_cs = gru_layer(0, x_cs, h0s, hown0)
                hp1, h1p_cs = gru_layer(1, h0p_cs, h1s, hown1)
                return hp0, hp1, h0p_cs, h1p_cs

            def stream_and_softmax(xf):
                # vocab projection: stream weight tiles as the moving operand.
                # lhsT packs (xh, xl) as an M=2 stationary so one Whi stream
                # yields both products; the xh*Wlo correction accumulates into
                # row 0; a K=2 ones-matmul then sums the two psum rows.
                xh = acts.tile([P, KC], bf16, tag="xh")
                nc.vector.tensor_copy(xh, xf)
                xl = acts.tile([P, KC], bf16, tag="xl")
                nc.vector.tensor_sub(xl, xf, xh)
                x2 = acts.tile([P, KC, 2], bf16, tag="x2")
                nc.vector.tensor_copy(x2[:, :, 0], xh)
                nc.vector.tensor_copy(x2[:, :, 1], xl)
                logits_row = rowbig.tile([1, VPAD], f32, tag="lrow")
                s_parts = tmp.tile([1, NV], f32, tag="sparts")
                for v in range(nv):
                    w = VT if v < NV - 1 else VTAIL
                    pv = ps_big.tile([2, VT], f32, tag="psv")
                    for kc in range(KC):
                        wtile = stream.tile([P, 2 * VT], bf16, tag="wt")
                        if w == VT:
                            nc.sync.dma_start(out=wtile, in_=wt[v, kc, :, :])
                        else:
                            nc.sync.dma_start(
                                out=wtile[:, :w], in_=wt[v, kc, :, :w]
                            )
                            nc.sync.dma_start(
                                out=wtile[:, VT : VT + w],
                                in_=wt[v, kc, :, VT : VT + w],
                            )
                        nc.tensor.matmul(
                            pv[:, :w], x2[:, kc, :], wtile[:, :w],
                            start=(kc == 0), stop=False,
                        )
                        nc.tensor.matmul(
                            pv[0:1, :w], xh[:, kc : kc + 1],
                            wtile[:, VT : VT + w],
                            start=False, stop=False,
                        )
                    # + out_b via K=1 matmuls against the bf16 hi/lo bias rows
                    nc.tensor.matmul(
                        pv[0:1, :w],
                        ones_bf[0:1, 0:1],
                        outb_sb[0:1, v * VT : v * VT + w],
                        start=False,
                        stop=False,
                    )
                    nc.tensor.matmul(
                        pv[0:1, :w],
                        ones_bf[0:1, 0:1],
                        outb_sb[0:1, VPAD + v * VT : VPAD + v * VT + w],
                        start=False,
                        stop=True,
                    )
                    sb2 = rows.tile([2, VT], f32, tag="row2")
                    nc.scalar.copy(out=sb2[:, :w], in_=pv[:, :w])
                    pv1 = ps_one.tile([1, VT], f32, tag="psv1")
                    nc.tensor.matmul(
                        pv1[:, :w], ones_col[0:2, :], sb2[:, :w],
                        start=True, stop=True,
                    )
                    nc.scalar.copy(
                        out=logits_row[:, v * VT : v * VT + w], in_=pv1[:, :w]
                    )
                    # exp + free-dim sum (hidden under the DMA-bound stream)
                    e_row = rows.tile([1, VT], f32, tag="erow")
                    nc.scalar.activation(
                        out=e_row[:, :w], in_=pv1[:, :w], func=AF.Exp, scale=1.0,
                        accum_out=s_parts[:, v : v + 1],
                    )

                # log-sum-exp over the full vocab, then normalize
                S_sb = tmp.tile([1, 1], f32, tag="S")
                nc.vector.tensor_reduce(
                    S_sb, s_parts[:, :nv], mybir.AxisListType.X,
                    mybir.AluOpType.add,
                )
                ar_in = dram.tile([1, 1], f32, tag="ar_in")
                ar_out = dram.tile([1, 1], f32, tag="ar_out")
                nc.sync.dma_start(out=ar_in, in_=S_sb)
                if use_collectives:
                    nc.gpsimd.collective_compute(
                        "AllReduce",
                        mybir.AluOpType.add,
                        replica_groups=groups,
                        ins=[ar_in.opt()],
                        outs=[ar_out.opt()],
                    )
                else:
                    nc.sync.dma_start(out=ar_out, in_=ar_in)
                St_sb = tmp.tile([1, 1], f32, tag="St")
                nc.sync.dma_start(out=St_sb, in_=ar_out)
                lse_sb = tmp.tile([1, 1], f32, tag="lse")
                nc.scalar.activation(out=lse_sb, in_=St_sb, func=AF.Ln, scale=1.0)
                # in-place exact row subtract on DVE (partition 0)
                nc.vector.tensor_scalar_sub(logits_row, logits_row, lse_sb)
                nc.sync.dma_start(out=o_log, in_=logits_row)

            h0s, h1s = h0_sb, h1_sb
            hown0, hown1 = hown_sb[:, 0:1], hown_sb[:, 1:2]
            for _r in range(rounds):
                if skip_chain:
                    xf = h1s
                else:
                    hp0, hp1, h0p_cs, h1p_cs = chain(h0s, h1s, hown0, hown1)
                    xf = h1p_cs  # final hidden, feeds the vocab projection
                    h0s, h1s = h0p_cs, h1p_cs
                    hown0, hown1 = hp0, hp1
                stream_and_softmax(xf)

    nc.compile()
    return nc


def _prepare_in_maps(inputs):
    f = np.float32
    ix = np.asarray(inputs["input"]).reshape(-1)
    idx = int(ix[0])
    hidden = np.ascontiguousarray(np.asarray(inputs["hidden"], dtype=f))
    enc_np = np.ascontiguousarray(np.asarray(inputs["encoder_outputs"], dtype=f))
    emb_W = np.asarray(inputs["emb_W"])
    attn_W = np.asarray(inputs["attn_W"], dtype=f)
    attn_b = np.asarray(inputs["attn_b"], dtype=f)
    comb_W = np.asarray(inputs["comb_W"], dtype=f)
    comb_b = np.asarray(inputs["comb_b"], dtype=f)
    gru_Wih = np.asarray(inputs["gru_Wih"], dtype=f)
    gru_Whh = np.asarray(inputs["gru_Whh"], dtype=f)
    gru_bih = np.asarray(inputs["gru_bih"], dtype=f)
    gru_bhh = np.asarray(inputs["gru_bhh"], dtype=f)
    out_W = np.asarray(inputs["out_W"], dtype=f)
    out_b = np.asarray(inputs["out_b"], dtype=f)

    emb_row = np.asarray(emb_W[idx], dtype=f)                      # (H,)
    emb_cs = np.ascontiguousarray(emb_row.reshape(KC, P).T)        # (128, 8)
    h0_cs = np.ascontiguousarray(hidden[0, 0].reshape(KC, P).T)
    h1_cs = np.ascontiguousarray(hidden[1, 0].reshape(KC, P).T)
    enc_host = np.ascontiguousarray(enc_np.reshape(ML, KC, P))     # (50, 8, 128)
    aw_host = np.ascontiguousarray(
        attn_W.T.reshape(2 * KC, P, ML).transpose(1, 0, 2)         # (128, 16, 50)
    )
    ab_host = np.ascontiguousarray(attn_b.reshape(ML, 1))

    combT = comb_W.T                                               # (2H, H)
    gruT_ih = [gru_Wih[l].T for l in range(L)]                     # (H, 3H)
    gruT_hh = [gru_Whh[l].T for l in range(L)]

    in_maps = []
    for j in range(M):
        sj = slice(P * j, P * (j + 1))
        cw_host = np.ascontiguousarray(
            combT[:, sj].reshape(2 * KC, P, P).transpose(1, 0, 2)  # (128, 16, 128)
        )
        cb_host = np.ascontiguousarray(comb_b[sj].reshape(P, 1))
        gw_host = np.empty((P, L * 2 * 3 * KC, P), dtype=f)
        gb_host = np.empty((P, L, 4), dtype=f)
        for l in range(L):
            for mat, T in ((0, gruT_ih[l]), (1, gruT_hh[l])):
                for g in range(3):
                    cols = slice(g * H + P * j, g * H + P * (j + 1))
                    blk = T[:, cols].reshape(KC, P, P).transpose(1, 0, 2)
                    base = ((l * 2 + mat) * 3 + g) * KC
                    gw_host[:, base : base + KC, :] = blk
            gb_host[:, l, 0] = gru_bih[l, 0 * H + P * j : 0 * H + P * (j + 1)] + \
                gru_bhh[l, 0 * H + P * j : 0 * H + P * (j + 1)]
            gb_host[:, l, 1] = gru_bih[l, 1 * H + P * j : 1 * H + P * (j + 1)] + \
                gru_bhh[l, 1 * H + P * j : 1 * H + P * (j + 1)]
            gb_host[:, l, 2] = gru_bih[l, 2 * H + P * j : 2 * H + P * (j + 1)]
            gb_host[:, l, 3] = gru_bhh[l, 2 * H + P * j : 2 * H + P * (j + 1)]
        hown_host = np.ascontiguousarray(hidden[:, 0, sj].T)       # (128, 2)

        wsT = out_W[VS * j : VS * (j + 1)].T                       # (H, 6250)
        wsT_pad = np.zeros((H, VPAD), dtype=f)
        wsT_pad[:, :VS] = wsT
        import ml_dtypes

        bf = ml_dtypes.bfloat16
        hi = wsT_pad.astype(bf)
        lo = (wsT_pad - hi.astype(f)).astype(bf)
        wt_host = np.empty((NV, KC, P, 2 * VT), dtype=bf)
        wt_host[..., :VT] = hi.reshape(KC, P, NV, VT).transpose(2, 0, 1, 3)
        wt_host[..., VT:] = lo.reshape(KC, P, NV, VT).transpose(2, 0, 1, 3)
        ob = np.zeros((VPAD,), dtype=f)
        ob[:VS] = out_b[VS * j : VS * (j + 1)]
        outb_host = np.zeros((1, 2 * VPAD), dtype=bf)
        outb_host[0, :VPAD] = ob.astype(bf)
        outb_host[0, VPAD:] = (ob - outb_host[0, :VPAD].astype(f)).astype(bf)

        in_maps.append(
            {
                "wt": wt_host,
                "gw": gw_host,
                "gb": gb_host,
                "cw": cw_host,
                "cb": cb_host,
                "aw": aw_host,
                "ab": ab_host,
                "enc": enc_host,
                "emb": emb_cs,
                "h0": h0_cs,
                "h1": h1_cs,
                "hown": hown_host,
                "outb": outb_host,
            }
        )
    return in_maps


def _assemble(results):
    logits = np.concatenate(
        [results[j]["out_logits"][0, :VS] for j in range(M)]
    )
    out = logits.reshape(1, V).astype(np.float32)
    newh = np.stack(
        [
            np.concatenate([results[j]["out_newh"][l] for j in range(M)])
            for l in range(L)
        ]
    ).reshape(L, 1, H).astype(np.float32)
    attw = results[0]["out_attw"].reshape(1, ML).astype(np.float32)
    return out, newh, attw


def _get_prog():
    global _prog
    if _prog is None:
        _prog = _build()
    return _prog


def kernel(**inputs):
    from concourse.bass_utils import run_bass_kernel_spmd

    nc = _get_prog()
    in_maps = _prepare_in_maps(inputs)
    res = run_bass_kernel_spmd(nc, in_maps, list(range(M)))
    return _assemble(res.results)


def run_traced(inputs, **kw):
    """Like kernel() but returns (outputs, BassKernelResults) with tracing on."""
    from concourse.bass_utils import run_bass_kernel_spmd

    nc = _get_prog()
    in_maps = _prepare_in_maps(inputs)
    res = run_bass_kernel_spmd(nc, in_maps, list(range(M)), trace=True, **kw)
    return _assemble(res.results), res
